# revision 31
# baseline (speedup 1.0000x reference)
"""LIF spiking-neuron scan kernel for Trainium2 (Bass/Tile), 8-core SPMD.

Reference semantics (per element, T=4 sequential steps):
    mem = 0
    for t in range(T):
        mem = mem + x[t]
        s[t] = (mem >= 1.0)          # spike, exact 0.0/1.0 fp32
        mem = mem * (mem < 1.0)      # hard reset on spike
All membrane math is fp32 and bit-exact vs the jax reference.

Sharding: x is [T*B, C, H, W] = [256, 128, 32, 32] fp32. Reshaped to
[T=4, B=64, C*H*W]; B is data-parallel sharded 8 ways (8 batch rows per
core). Each core's shard is viewed as [T, 128, 8192] fp32. The T-scan is
local per core; no communication.

Per-core engine plan (memory-bound target, DMA floor ~59 us):
  - DMA: 16 MiB in (fp32) + 4 MiB out (uint8 spikes, host converts to
    fp32) across 16 HW queue-engines @ ~358 GB/s aggregate.
  - Work per (t, chunk) cell: u = mem + x_t; s = (u >= 1) as uint8
    (DVE 2x mode); mem = (u < 1) * u (fused scalar_tensor_tensor).
  - The adds are split between DVE and GpSimd (Pool) per `add_plan` so
    no engine exceeds the DMA floor; is_ge + reset stay on DVE.
"""

import numpy as np

import concourse.bacc as bacc
import concourse.mybir as mybir
import concourse.tile as tile
from concourse.bass_utils import run_bass_kernel_spmd

T = 4
B = 64
CHW = 128 * 32 * 32  # 131072
N_CORES = 8
B_SHARD = B // N_CORES           # 8
ELEMS = B_SHARD * CHW            # 1048576 elems per timestep per core
P = 128
F = ELEMS // P                   # 8192

_cache = {}


def _build_module(f_tile=2048, x_bufs=4, s_bufs=6, out_dt="uint8",
                  t_major=False, add_plan=None, isge_plan=None,
                  stt_plan=None, split_dma=True, u_psum=False, u_bufs=2,
                  stt_from_s=False, spike_act=False, chunks=None,
                  u_fresh=False, rev_last=False, store_per_level=False,
                  chunks_t0=None):
    """add_plan/isge_plan/stt_plan: strings of 'v' (DVE) / 'p' (Pool),
    one char per cell in emission order; None = all 'v'.
    u_psum: stage u = mem + x_t in PSUM to take the intermediate's
    read/write traffic off SBUF (SBUF bandwidth is the roofline when
    DVE and Pool run concurrently).
    spike_act: compute the spike on the (otherwise idle) Activation
    engine as sgn = Sign(u - 1) in int8 {-1,0,1}; the host maps
    s = (sgn >= 0). Exact: u-1 of fp32 values near 1 is never denormal
    (spacing 2^-24), and u==1 gives Sign(+0) in {0,1}, both spike."""
    if chunks is None:
        chunks = [f_tile] * (F // f_tile)
    assert sum(chunks) == F, chunks
    n_j = len(chunks)
    col0 = [sum(chunks[:i]) for i in range(n_j)]
    # Optional finer chunking for the t=0 level only (faster pipeline
    # ramp: the first compute waits on a smaller first DMA). Uses a
    # single shared mem tile with subtile deps so granularities can
    # differ between levels.
    if chunks_t0 is not None:
        assert sum(chunks_t0) == F, chunks_t0
    lvl_chunks = {t: (chunks_t0 if t == 0 and chunks_t0 else chunks)
                  for t in range(T)}
    lvl_col0 = {t: [sum(c[:i]) for i in range(len(c))]
                for t, c in lvl_chunks.items()}
    mem_shared = chunks_t0 is not None
    if spike_act:
        out_dt = "int8"
    odt = getattr(mybir.dt, out_dt)
    fp32 = mybir.dt.float32
    Alu = mybir.AluOpType

    nc = bacc.Bacc("TRN2", target_bir_lowering=False, debug=False)
    x = nc.dram_tensor("x", (T, P, F), fp32, kind="ExternalInput").ap()
    out = nc.dram_tensor("out", (T, P, F), odt, kind="ExternalOutput").ap()

    def eng(plan, idx):
        if plan is None:
            return nc.vector
        return nc.gpsimd if plan[idx % len(plan)] == "p" else nc.vector

    with tile.TileContext(nc) as tc:
        with (
            tc.tile_pool(name="xp", bufs=x_bufs) as xpool,
            tc.tile_pool(name="sp", bufs=s_bufs) as spool,
            tc.tile_pool(name="mp", bufs=n_j) as mpool,
            tc.tile_pool(name="up", bufs=u_bufs,
                         space="PSUM" if u_psum else "SBUF") as upool,
        ):
            if t_major:
                order = []
                for t in range(T):
                    js = range(len(lvl_chunks[t]))
                    if rev_last and t == T - 1:
                        js = reversed(js)
                    order += [(t, j) for j in js]
            else:
                assert chunks_t0 is None
                order = [(t, j) for j in range(n_j) for t in range(T)]
            if spike_act:
                neg1 = mpool.tile([P, 1], fp32, tag="neg1", bufs=1)
                nc.vector.memset(neg1[:], -1.0)
            mems = {}
            if mem_shared:
                mem_all = mpool.tile([P, F], fp32, tag="mem", bufs=1)
            n_add = n_isge = n_stt = 0
            s_lvls = {}
            done_in_lvl = {}
            for t, j in order:
                w = lvl_chunks[t][j]
                c0 = lvl_col0[t][j]
                sl = slice(c0, c0 + w)
                xt = xpool.tile([P, w], fp32, tag="x")
                nc.sync.dma_start(out=xt[:], in_=x[t, :, sl])
                if mem_shared:
                    mem_sl = mem_all[:, sl]
                else:
                    if t == 0:
                        mem = mpool.tile([P, w], fp32, tag=f"mem{j}", bufs=1)
                        mems[j] = mem
                    mem_sl = mems[j][:]
                if t == 0:
                    u = xt
                else:
                    if u_psum or u_fresh:
                        # Fresh u tile per cell: the in-place variant
                        # makes add(j,t+1) wait for the act engine's
                        # sign(j,t) read of mems[j] (WAR); a rotating u
                        # tile removes that cross-engine coupling.
                        u = upool.tile([P, w], fp32, tag="u")
                        eng(add_plan, n_add).tensor_add(
                            u[:], mem_sl, xt[:])
                    else:
                        eng(add_plan, n_add).tensor_add(
                            mem_sl, mem_sl, xt[:])
                        u = mem_sl
                    n_add += 1
                if store_per_level:
                    if t not in s_lvls:
                        s_lvl = spool.tile([P, F], odt, tag="s")
                        s_lvls[t] = s_lvl
                        done_in_lvl[t] = 0
                    s = s_lvls[t][:, sl]
                else:
                    s = spool.tile([P, w], odt, tag="s")
                if spike_act:
                    nc.scalar.sign(s[:], u[:], bias=neg1[:])
                else:
                    eng(isge_plan, n_isge).tensor_scalar(
                        s[:], u[:], 1.0, None, Alu.is_ge)
                    n_isge += 1
                if t < T - 1:
                    if spike_act and u_psum:
                        # mem' = (sgn < 0) * u; single PSUM read (u).
                        eng(stt_plan, n_stt).scalar_tensor_tensor(
                            mem_sl, s[:], 0.0, u[:],
                            Alu.is_lt, Alu.mult)
                    elif stt_from_s:
                        # mem' = (s == 0) * u  — exact for s in {0,1};
                        # reads s (1B) instead of a second u read, and
                        # keeps the PSUM-read count at one.
                        eng(stt_plan, n_stt).scalar_tensor_tensor(
                            mem_sl, s[:], 0.0, u[:],
                            Alu.is_equal, Alu.mult)
                    else:
                        eng(stt_plan, n_stt).scalar_tensor_tensor(
                            mem_sl, u[:], 1.0, u[:], Alu.is_lt, Alu.mult)
                    n_stt += 1
                # Stores go out on the Activation HWDGE queue so a store
                # waiting on its is_ge never blocks x prefetches behind
                # it in the (in-order) SP queue.
                out_q = nc.scalar if split_dma else nc.sync
                if store_per_level:
                    done_in_lvl[t] += 1
                    if done_in_lvl[t] == n_j:
                        out_q.dma_start(out=out[t, :, :], in_=s_lvls[t][:])
                else:
                    out_q.dma_start(out=out[t, :, sl], in_=s[:])
    nc.compile()
    return nc


# Best hardware sweep result (~74 us/core vs 91 us baseline):
# spike on the Activation engine (int8 sgn, host maps >= 0), t-major
# emission for cross-engine pipelining, fresh u tiles to break the
# add->sign WAR coupling, deep x prefetch, loads on SP / stores on
# Activation HWDGE queues.
BEST = {
    "spike_act": True,
    "t_major": True,
    "x_bufs": 10,
    "u_fresh": True,
    "u_bufs": 4,
}


def _get_module():
    if "nc" not in _cache:
        _cache["nc"] = _build_module(**BEST)
    return _cache["nc"]


def _shard_inputs(x_np):
    # x_np: [T*B, C, H, W] fp32 -> per-core [T, P, F]
    xr = np.ascontiguousarray(x_np).reshape(T, B, CHW)
    shards = []
    for k in range(N_CORES):
        sh = np.ascontiguousarray(xr[:, k * B_SHARD : (k + 1) * B_SHARD]).reshape(
            T, P, F
        )
        shards.append(sh)
    return shards


def _unshard_outputs(outs):
    # outs: list of [T, P, F] (uint8 or fp32) -> [T*B, C, H, W] fp32
    full = np.empty((T, B, CHW), dtype=np.float32)
    for k, o in enumerate(outs):
        o = o.reshape(T, B_SHARD, CHW)
        if o.dtype == np.int8:
            # spike_act mode: device stored sgn(u-1) in {-1,0,1};
            # spike = (sgn >= 0).
            full[:, k * B_SHARD : (k + 1) * B_SHARD] = o >= 0
        else:
            full[:, k * B_SHARD : (k + 1) * B_SHARD] = o
    return full.reshape(T * B, 128, 32, 32)


def kernel(x, T=4, **_unused):
    x_np = np.asarray(x, dtype=np.float32)
    assert int(T) == 4, f"kernel hardcoded for T=4, got {T}"
    assert x_np.shape == (256, 128, 32, 32), x_np.shape

    nc = _get_module()
    shards = _shard_inputs(x_np)
    in_maps = [{"x": sh} for sh in shards]
    res = run_bass_kernel_spmd(nc, in_maps, list(range(N_CORES)))
    outs = [r["out"] for r in res.results]
    return _unshard_outputs(outs)


# revision 37
# speedup vs baseline: 1.0165x; 1.0165x over previous
"""LIF spiking-neuron scan kernel for Trainium2 (Bass/Tile), 8-core SPMD.

Reference semantics (per element, T=4 sequential steps):
    mem = 0
    for t in range(T):
        mem = mem + x[t]
        s[t] = (mem >= 1.0)          # spike, exact 0.0/1.0 fp32
        mem = mem * (mem < 1.0)      # hard reset on spike
All membrane math is fp32 and bit-exact vs the jax reference.

Sharding: x is [T*B, C, H, W] = [256, 128, 32, 32] fp32. Reshaped to
[T=4, B=64, C*H*W]; B is data-parallel sharded 8 ways (8 batch rows per
core). Each core's shard is viewed as [T, 128, 8192] fp32. The T-scan is
local per core; no communication.

Per-core engine plan (memory-bound target; ~74 us measured, DMA floor
~59 us + ~13 us fixed Tile init/finalize):
  - DMA: 16 MiB in (fp32) + 4 MiB out (int8 sgn, host maps to fp32
    spikes) across 16 HW queue-engines @ ~358 GB/s aggregate. Loads on
    the SP HWDGE queue, stores on the Activation HWDGE queue so a
    store waiting on compute never blocks prefetches (both in-order).
  - DVE per (t, chunk) cell: u = mem + x_t (tensor_tensor, into a
    fresh rotating u tile to avoid WAR coupling with the act engine),
    and mem' = (u < 1) * u (fused scalar_tensor_tensor). ~55 us busy.
  - Activation engine (runs concurrently with DVE at full speed,
    unlike GpSimd which thrashes SBUF): sgn = Sign(u - 1) as int8
    {-1,0,1}; host computes spike = (sgn >= 0). Exact: fp32 sums near
    1.0 are never denormal (spacing 2^-24), u == 1.0 gives Sign(+0)
    in {0,1} and both map to spike.
  - t-major emission so the per-chunk t-scan chains pipeline across
    the in-order engine queues.
"""

import numpy as np

import concourse.bacc as bacc
import concourse.mybir as mybir
import concourse.tile as tile
from concourse.bass_utils import run_bass_kernel_spmd

T = 4
B = 64
CHW = 128 * 32 * 32  # 131072
N_CORES = 8
B_SHARD = B // N_CORES           # 8
ELEMS = B_SHARD * CHW            # 1048576 elems per timestep per core
P = 128
F = ELEMS // P                   # 8192

_cache = {}


def _build_module(f_tile=2048, x_bufs=4, s_bufs=6, out_dt="uint8",
                  t_major=False, add_plan=None, isge_plan=None,
                  stt_plan=None, split_dma=True, u_psum=False, u_bufs=2,
                  stt_from_s=False, spike_act=False, chunks=None,
                  u_fresh=False, rev_last=False, store_per_level=False,
                  chunks_t0=None, last_sgn_dve=0, t0_sub=1):
    """add_plan/isge_plan/stt_plan: strings of 'v' (DVE) / 'p' (Pool),
    one char per cell in emission order; None = all 'v'.
    u_psum: stage u = mem + x_t in PSUM to take the intermediate's
    read/write traffic off SBUF (SBUF bandwidth is the roofline when
    DVE and Pool run concurrently).
    spike_act: compute the spike on the (otherwise idle) Activation
    engine as sgn = Sign(u - 1) in int8 {-1,0,1}; the host maps
    s = (sgn >= 0). Exact: u-1 of fp32 values near 1 is never denormal
    (spacing 2^-24), and u==1 gives Sign(+0) in {0,1}, both spike."""
    if chunks is None:
        chunks = [f_tile] * (F // f_tile)
    assert sum(chunks) == F, chunks
    n_j = len(chunks)
    col0 = [sum(chunks[:i]) for i in range(n_j)]
    # Optional finer chunking for the t=0 level only (faster pipeline
    # ramp: the first compute waits on a smaller first DMA). Uses a
    # single shared mem tile with subtile deps so granularities can
    # differ between levels.
    if chunks_t0 is not None:
        assert sum(chunks_t0) == F, chunks_t0
        assert t0_sub == 1
    lvl_chunks = {t: (chunks_t0 if t == 0 and chunks_t0 else chunks)
                  for t in range(T)}
    lvl_col0 = {t: [sum(c[:i]) for i in range(len(c))]
                for t, c in lvl_chunks.items()}
    mem_shared = chunks_t0 is not None
    if spike_act:
        out_dt = "int8"
    odt = getattr(mybir.dt, out_dt)
    fp32 = mybir.dt.float32
    Alu = mybir.AluOpType

    nc = bacc.Bacc("TRN2", target_bir_lowering=False, debug=False)
    x = nc.dram_tensor("x", (T, P, F), fp32, kind="ExternalInput").ap()
    out = nc.dram_tensor("out", (T, P, F), odt, kind="ExternalOutput").ap()

    def eng(plan, idx):
        if plan is None:
            return nc.vector
        return nc.gpsimd if plan[idx % len(plan)] == "p" else nc.vector

    with tile.TileContext(nc) as tc:
        with (
            tc.tile_pool(name="xp", bufs=x_bufs) as xpool,
            tc.tile_pool(name="sp", bufs=s_bufs) as spool,
            tc.tile_pool(name="mp", bufs=n_j) as mpool,
            tc.tile_pool(name="up", bufs=u_bufs,
                         space="PSUM" if u_psum else "SBUF") as upool,
        ):
            if t_major:
                order = []
                for t in range(T):
                    js = range(len(lvl_chunks[t]))
                    if rev_last and t == T - 1:
                        js = reversed(js)
                    order += [(t, j) for j in js]
            else:
                assert chunks_t0 is None
                order = [(t, j) for j in range(n_j) for t in range(T)]
            if spike_act:
                neg1 = mpool.tile([P, 1], fp32, tag="neg1", bufs=1)
                nc.vector.memset(neg1[:], -1.0)
            mems = {}
            if mem_shared:
                mem_all = mpool.tile([P, F], fp32, tag="mem", bufs=1)
            n_add = n_isge = n_stt = 0
            s_lvls = {}
            done_in_lvl = {}
            # Expand t=0 cells into t0_sub sub-slices of each chunk so
            # the first compute only waits on a fraction of the first
            # chunk's DMA (faster pipeline ramp).
            cells = []
            for t, j in order:
                w = lvl_chunks[t][j]
                c0 = lvl_col0[t][j]
                if t == 0 and t0_sub > 1:
                    sw = w // t0_sub
                    for k in range(t0_sub):
                        kw = sw if k < t0_sub - 1 else w - sw * (t0_sub - 1)
                        cells.append((t, j, c0 + k * sw, kw))
                else:
                    cells.append((t, j, c0, w))
            for ci, (t, j, c0, w) in enumerate(cells):
                sl = slice(c0, c0 + w)
                xt = xpool.tile([P, w], fp32, tag="x")
                nc.sync.dma_start(out=xt[:], in_=x[t, :, sl])
                if mem_shared:
                    mem_sl = mem_all[:, sl]
                else:
                    if t == 0 and j not in mems:
                        mem = mpool.tile([P, lvl_chunks[t][j]], fp32,
                                         tag=f"mem{j}", bufs=1)
                        mems[j] = mem
                    r0 = c0 - col0[j]
                    mem_sl = mems[j][:, r0:r0 + w]
                if t == 0:
                    u = xt
                else:
                    if u_psum or u_fresh:
                        # Fresh u tile per cell: the in-place variant
                        # makes add(j,t+1) wait for the act engine's
                        # sign(j,t) read of mems[j] (WAR); a rotating u
                        # tile removes that cross-engine coupling.
                        u = upool.tile([P, w], fp32, tag="u")
                        eng(add_plan, n_add).tensor_add(
                            u[:], mem_sl, xt[:])
                    else:
                        eng(add_plan, n_add).tensor_add(
                            mem_sl, mem_sl, xt[:])
                        u = mem_sl
                    n_add += 1
                if store_per_level:
                    if t not in s_lvls:
                        s_lvl = spool.tile([P, F], odt, tag="s")
                        s_lvls[t] = s_lvl
                        done_in_lvl[t] = 0
                    s = s_lvls[t][:, sl]
                else:
                    s = spool.tile([P, w], odt, tag="s")
                if spike_act:
                    if last_sgn_dve and ci >= len(cells) - last_sgn_dve:
                        # Tail trim: final cells' spike on DVE as
                        # (u < 1) * -1 -> {-1, 0} int8 (2x mode); the
                        # host's (sgn >= 0) map reads it identically,
                        # and the last store no longer waits on the
                        # act engine's lag behind DVE.
                        nc.vector.tensor_scalar(
                            s[:], u[:], 1.0, -1.0, Alu.is_lt, Alu.mult)
                    else:
                        nc.scalar.sign(s[:], u[:], bias=neg1[:])
                else:
                    eng(isge_plan, n_isge).tensor_scalar(
                        s[:], u[:], 1.0, None, Alu.is_ge)
                    n_isge += 1
                if t < T - 1:
                    if spike_act and u_psum:
                        # mem' = (sgn < 0) * u; single PSUM read (u).
                        eng(stt_plan, n_stt).scalar_tensor_tensor(
                            mem_sl, s[:], 0.0, u[:],
                            Alu.is_lt, Alu.mult)
                    elif stt_from_s:
                        # mem' = (s == 0) * u  — exact for s in {0,1};
                        # reads s (1B) instead of a second u read, and
                        # keeps the PSUM-read count at one.
                        eng(stt_plan, n_stt).scalar_tensor_tensor(
                            mem_sl, s[:], 0.0, u[:],
                            Alu.is_equal, Alu.mult)
                    else:
                        eng(stt_plan, n_stt).scalar_tensor_tensor(
                            mem_sl, u[:], 1.0, u[:], Alu.is_lt, Alu.mult)
                    n_stt += 1
                # Stores go out on the Activation HWDGE queue so a store
                # waiting on its is_ge never blocks x prefetches behind
                # it in the (in-order) SP queue.
                out_q = nc.scalar if split_dma else nc.sync
                if store_per_level:
                    done_in_lvl[t] += 1
                    if done_in_lvl[t] == n_j:
                        out_q.dma_start(out=out[t, :, :], in_=s_lvls[t][:])
                else:
                    out_q.dma_start(out=out[t, :, sl], in_=s[:])
    nc.compile()
    return nc


# Best hardware sweep result (~74 us/core vs 91 us baseline):
# spike on the Activation engine (int8 sgn, host maps >= 0), t-major
# emission for cross-engine pipelining, fresh u tiles to break the
# add->sign WAR coupling, deep x prefetch, loads on SP / stores on
# Activation HWDGE queues.
BEST = {
    "spike_act": True,
    "t_major": True,
    "x_bufs": 10,
    "u_fresh": True,
    "u_bufs": 4,
    # Final cell's spike on DVE as (u<1)*-1 -> {-1,0}: the closing
    # store stops waiting on the act engine's lag behind DVE (~-1.3us).
    "last_sgn_dve": 1,
}


def _get_module():
    if "nc" not in _cache:
        _cache["nc"] = _build_module(**BEST)
    return _cache["nc"]


def _shard_inputs(x_np):
    # x_np: [T*B, C, H, W] fp32 -> per-core [T, P, F]
    xr = np.ascontiguousarray(x_np).reshape(T, B, CHW)
    shards = []
    for k in range(N_CORES):
        sh = np.ascontiguousarray(xr[:, k * B_SHARD : (k + 1) * B_SHARD]).reshape(
            T, P, F
        )
        shards.append(sh)
    return shards


def _unshard_outputs(outs):
    # outs: list of [T, P, F] (uint8 or fp32) -> [T*B, C, H, W] fp32
    full = np.empty((T, B, CHW), dtype=np.float32)
    for k, o in enumerate(outs):
        o = o.reshape(T, B_SHARD, CHW)
        if o.dtype == np.int8:
            # spike_act mode: device stored sgn(u-1) in {-1,0,1};
            # spike = (sgn >= 0).
            full[:, k * B_SHARD : (k + 1) * B_SHARD] = o >= 0
        else:
            full[:, k * B_SHARD : (k + 1) * B_SHARD] = o
    return full.reshape(T * B, 128, 32, 32)


def kernel(x, T=4, **_unused):
    x_np = np.asarray(x, dtype=np.float32)
    assert int(T) == 4, f"kernel hardcoded for T=4, got {T}"
    assert x_np.shape == (256, 128, 32, 32), x_np.shape

    nc = _get_module()
    shards = _shard_inputs(x_np)
    in_maps = [{"x": sh} for sh in shards]
    res = run_bass_kernel_spmd(nc, in_maps, list(range(N_CORES)))
    outs = [r["out"] for r in res.results]
    return _unshard_outputs(outs)
